# revision 2
# baseline (speedup 1.0000x reference)
"""Trainium2 Bass kernel for nn_Diversity2 (per-row Pearson correlation of
temperature softmaxes, averaged).

Math per row r of x1, x2 [N, C]: corr is invariant to per-row affine
transforms of p = softmax(x/T), so with e = exp(x/T), d = e - 1:
    Z' = sum_c d,  S11 = sum d1^2, S22 = sum d2^2, S12 = sum d1 d2
    corr = (S12 - Z1'Z2'/C) / sqrt((S11 - Z1'^2/C)(S22 - Z2'^2/C))
Answer = SCALE * mean_r(corr). Final stats -> host, tail math in fp64.

Key speed levers vs the fp32 version:
  - inputs shipped fp16 (halves HBM traffic; rel-err contribution ~1e-4)
  - one merged ACT Exp per tile over [P, 2, 1000] (x1|x2 interleaved),
    fp16 out, accum -> Z1+Z2
  - DVE in all-16-bit mode: ts (4x) materializes d1 = e1-1 (exact in fp16
    by Sterbenz, since e in [0.78, 1.33]); three stt passes (2x) accumulate
    S11, A12 = S12+Z1', A22 = S22+Z2'. A fraction of A22 passes moves to
    ACT as Square(e2 - 1) to balance engines (tunable ACT_S22_SET).

Per-core: 8192 rows = 64 tiles of [128, 2*1000] fp16. Output: 5 stat
arrays [128, 64] fp32 packed as one [128, 320] tensor.
"""

import sys

if "/opt/trn_rl_repo" not in sys.path:
    sys.path.insert(0, "/opt/trn_rl_repo")

import numpy as np

T_INV = 1.0 / 20.0
SCALE = 0.3
N_ROWS = 65536
C = 1000
N_CORES = 8
P = 128
ROWS_PER_CORE = N_ROWS // N_CORES  # 8192
N_TILES = ROWS_PER_CORE // P  # 64

# tiles (i % 16) in this set compute S22 on ACT via Square(e2 - 1)
ACT_S22_SET = frozenset((2, 5, 8, 11, 14))

_PROG_CACHE: dict = {}


def build_program(n_tiles: int = N_TILES, num_devices: int = N_CORES):
    import concourse.tile as tile
    from concourse import bacc, mybir

    f16 = mybir.dt.float16
    f32 = mybir.dt.float32
    AF = mybir.ActivationFunctionType
    OP = mybir.AluOpType

    nc = bacc.Bacc(
        "TRN2", target_bir_lowering=False, debug=False, num_devices=num_devices
    )
    rows = n_tiles * P
    XM = nc.dram_tensor("xm", [rows, 2, C], f16, kind="ExternalInput").ap()
    OUT = nc.dram_tensor("out", [P, 5 * n_tiles], f32, kind="ExternalOutput").ap()

    with tile.TileContext(nc) as tc:
        with (
            tc.tile_pool(name="pin", bufs=6) as pin,
            tc.tile_pool(name="pe", bufs=4) as pe,
            tc.tile_pool(name="pd", bufs=3) as pd,
            tc.tile_pool(name="pscr", bufs=3) as pscr,
            tc.tile_pool(name="pstat", bufs=1) as pstat,
        ):
            neg1 = pstat.tile([P, 1], f32, tag="neg1")
            nc.vector.memset(neg1[:], -1.0)

            ZMIX = pstat.tile([P, n_tiles], f32, tag="zmix")
            Z1R = pstat.tile([P, n_tiles], f32, tag="z1r")
            S11A = pstat.tile([P, n_tiles], f32, tag="s11a")
            A12A = pstat.tile([P, n_tiles], f32, tag="a12a")
            A22A = pstat.tile([P, n_tiles], f32, tag="a22a")

            for i in range(n_tiles):
                xt = pin.tile([P, 2, C], f16, tag="xt")
                nc.sync.dma_start(out=xt[:], in_=XM[i * P : (i + 1) * P, :, :])

                # e = exp(x/T) for both tensors in one ACT pass;
                # accum -> Z1 + Z2 (= 2C + Z1' + Z2')
                em = pe.tile([P, 2, C], f16, tag="em")
                nc.scalar.activation(
                    em[:], xt[:], AF.Exp, scale=T_INV,
                    accum_out=ZMIX[:, i : i + 1],
                )
                e1 = em[:, 0, :]
                e2 = em[:, 1, :]

                # d1 = e1 - 1 exactly (Sterbenz); 4x-mode ts; accum -> Z1'
                d1 = pd.tile([P, C], f16, tag="d1")
                nc.vector.tensor_scalar(
                    d1[:], e1, -1.0, None, OP.add, OP.add,
                    accum_out=Z1R[:, i : i + 1],
                )

                # S11 = sum d1^2  (stt 2x mode)
                s_a = pscr.tile([P, C], f16, tag="s_a")
                nc.vector.scalar_tensor_tensor(
                    out=s_a[:], in0=d1[:], scalar=0.0, in1=d1[:],
                    op0=OP.add, op1=OP.mult,
                    accum_out=S11A[:, i : i + 1],
                )

                # A12 = sum d1*e2 = S12 + Z1'
                s_b = pscr.tile([P, C], f16, tag="s_b")
                nc.vector.scalar_tensor_tensor(
                    out=s_b[:], in0=d1[:], scalar=0.0, in1=e2,
                    op0=OP.add, op1=OP.mult,
                    accum_out=A12A[:, i : i + 1],
                )

                if (i % 16) in ACT_S22_SET:
                    # S22 directly on ACT: Square(e2 - 1), accum
                    s_c = pscr.tile([P, C], mybir.dt.bfloat16, tag="s_c")
                    nc.scalar.activation(
                        s_c[:], e2, AF.Square, bias=neg1[:],
                        accum_out=A22A[:, i : i + 1],
                    )
                else:
                    # A22 = sum (e2-1)*e2 = S22 + Z2'  (stt 2x mode)
                    s_d = pscr.tile([P, C], f16, tag="s_d")
                    nc.vector.scalar_tensor_tensor(
                        out=s_d[:], in0=e2, scalar=-1.0, in1=e2,
                        op0=OP.add, op1=OP.mult,
                        accum_out=A22A[:, i : i + 1],
                    )

            nt = n_tiles
            nc.sync.dma_start(out=OUT[:, 0 * nt : 1 * nt], in_=ZMIX[:])
            nc.sync.dma_start(out=OUT[:, 1 * nt : 2 * nt], in_=Z1R[:])
            nc.sync.dma_start(out=OUT[:, 2 * nt : 3 * nt], in_=S11A[:])
            nc.sync.dma_start(out=OUT[:, 3 * nt : 4 * nt], in_=A12A[:])
            nc.sync.dma_start(out=OUT[:, 4 * nt : 5 * nt], in_=A22A[:])

    nc.compile()
    return nc


def _get_program():
    key = "full"
    if key not in _PROG_CACHE:
        _PROG_CACHE[key] = build_program()
    return _PROG_CACHE[key]


def _host_tail(outs: list) -> float:
    """Combine per-core [P, 5*nt] stats into the final scalar in fp64."""
    nt = N_TILES
    total = 0.0
    dve_s22 = np.array([(i % 16) not in ACT_S22_SET for i in range(nt)])
    for o in outs:
        o = o.astype(np.float64)
        zmix = o[:, 0 * nt : 1 * nt]
        z1 = o[:, 1 * nt : 2 * nt]
        s11 = o[:, 2 * nt : 3 * nt]
        a12 = o[:, 3 * nt : 4 * nt]
        a22 = o[:, 4 * nt : 5 * nt]
        z2 = zmix - 2.0 * C - z1
        s12 = a12 - z1
        s22 = a22 - np.where(dve_s22[None, :], z2, 0.0)
        num = s12 - z1 * z2 / C
        b = s11 - z1 * z1 / C
        c = s22 - z2 * z2 / C
        total += (num / np.sqrt(b * c)).sum()
    return SCALE * total / float(N_ROWS)


def run_sharded(outputs1: np.ndarray, outputs2: np.ndarray, trace: bool = False):
    from concourse.bass_utils import run_bass_kernel_spmd

    nc = _get_program()
    n = outputs1.shape[0]
    xm = np.empty((n, 2, C), dtype=np.float16)
    xm[:, 0, :] = outputs1
    xm[:, 1, :] = outputs2
    in_maps = [
        {"xm": xm[i * ROWS_PER_CORE : (i + 1) * ROWS_PER_CORE]}
        for i in range(N_CORES)
    ]
    res = run_bass_kernel_spmd(nc, in_maps, list(range(N_CORES)), trace=trace)
    val = _host_tail([r["out"] for r in res.results])
    return np.asarray(val, dtype=np.float32), res


def kernel(outputs1, outputs2, targets=None, **_unused):
    val, _ = run_sharded(np.asarray(outputs1), np.asarray(outputs2))
    return val


# revision 4
# speedup vs baseline: 1.2130x; 1.2130x over previous
"""Trainium2 Bass kernel for nn_Diversity2 (per-row Pearson correlation of
temperature softmaxes, averaged).

Math per row r of x1, x2 [N, C]: corr is invariant to per-row affine
transforms of p = softmax(x/T), so with e = exp(x/T), d = e - 1:
    Z' = sum_c d, S11 = sum d1^2, S22 = sum d2^2, S12 = sum d1 d2
    corr = (S12 - Z1'Z2'/C) / sqrt((S11 - Z1'^2/C)(S22 - Z2'^2/C))
Answer = SCALE * mean_r(corr). Per-row stats -> host, tail math in fp64.

Hardware facts this design is built on (all measured on trn2):
  - every op with accum_out runs at 1x (~1120 ns DVE / ~1116+185 ns ACT
    per [128,1000] pass); ACT is dtype-independent (merged exp = 1851).
  - accumulators sum pre-rounding (fp32 internal) and the DVE accumulator
    has a small negative (truncation-like) bias at d-scale: ~-1.4e-5.
    The num path needs Z1' from a d-scale DVE accum so biases cancel;
    e-scale sums (ACT exp accum, ~1e-3 noise) are only safe in /C terms.
  - fp16 e output adds ~2e-3 relative error (within 2e-2 tolerance).

Balanced split, ~3340 ns/tile on each engine:
  ACT: merged Exp [P,2000] fp16 (accum -> Z1+Z2) + Square(e2-1) (accum -> S22)
  DVE: ts e1-1 (accum -> Z1', exact: Sterbenz), stt (-1+e1)*e1 -> S11+Z1',
       stt (-1+e1)*e2 -> S12+Z1'
"""

import sys

if "/opt/trn_rl_repo" not in sys.path:
    sys.path.insert(0, "/opt/trn_rl_repo")

import numpy as np

T_INV = 1.0 / 20.0
SCALE = 0.3
N_ROWS = 65536
C = 1000
N_CORES = 8
P = 128
ROWS_PER_CORE = N_ROWS // N_CORES  # 8192
N_TILES = ROWS_PER_CORE // P  # 64

_PROG_CACHE: dict = {}


def build_program(n_tiles: int = N_TILES, num_devices: int = N_CORES):
    import concourse.tile as tile
    from concourse import bacc, mybir

    f16 = mybir.dt.float16
    f32 = mybir.dt.float32
    bf16 = mybir.dt.bfloat16
    AF = mybir.ActivationFunctionType
    OP = mybir.AluOpType

    nc = bacc.Bacc(
        "TRN2", target_bir_lowering=False, debug=False, num_devices=num_devices
    )
    rows = n_tiles * P
    XM = nc.dram_tensor("xm", [rows, 2, C], f16, kind="ExternalInput").ap()
    OUT = nc.dram_tensor("out", [P, 5 * n_tiles], f32, kind="ExternalOutput").ap()

    with tile.TileContext(nc) as tc:
        with (
            tc.tile_pool(name="pin", bufs=6) as pin,
            tc.tile_pool(name="pe", bufs=4) as pe,
            tc.tile_pool(name="pscr", bufs=4) as pscr,
            tc.tile_pool(name="pstat", bufs=1) as pstat,
        ):
            neg1 = pstat.tile([P, 1], f32, tag="neg1")
            nc.vector.memset(neg1[:], -1.0)

            ZMIX = pstat.tile([P, n_tiles], f32, tag="zmix")
            Z1R = pstat.tile([P, n_tiles], f32, tag="z1r")
            S11A = pstat.tile([P, n_tiles], f32, tag="s11a")
            A12A = pstat.tile([P, n_tiles], f32, tag="a12a")
            S22A = pstat.tile([P, n_tiles], f32, tag="s22a")

            for i in range(n_tiles):
                xt = pin.tile([P, 2, C], f16, tag="xt")
                nc.sync.dma_start(out=xt[:], in_=XM[i * P : (i + 1) * P, :, :])

                # e = exp(x/T), both tensors; accum -> Z1+Z2 (e-scale, /C uses)
                em = pe.tile([P, 2, C], f16, tag="em")
                nc.scalar.activation(
                    em[:], xt[:], AF.Exp, scale=T_INV,
                    accum_out=ZMIX[:, i : i + 1],
                )
                e1 = em[:, 0, :]
                e2 = em[:, 1, :]

                # ACT: S22 = sum (e2-1)^2 via Square with bias -1
                s_c = pscr.tile([P, C], bf16, tag="s_c")
                nc.scalar.activation(
                    s_c[:], e2, AF.Square, bias=neg1[:],
                    accum_out=S22A[:, i : i + 1],
                )

                # DVE: Z1' = sum(e1-1) at d-scale (exact given e1: Sterbenz)
                s_z = pscr.tile([P, C], bf16, tag="s_z")
                nc.vector.tensor_scalar(
                    s_z[:], e1, -1.0, None, OP.add, OP.add,
                    accum_out=Z1R[:, i : i + 1],
                )
                # DVE: S11 + Z1' = sum (e1-1)*e1
                s_a = pscr.tile([P, C], bf16, tag="s_a")
                nc.vector.scalar_tensor_tensor(
                    out=s_a[:], in0=e1, scalar=-1.0, in1=e1,
                    op0=OP.add, op1=OP.mult,
                    accum_out=S11A[:, i : i + 1],
                )
                # DVE: S12 + Z1' = sum (e1-1)*e2
                s_b = pscr.tile([P, C], bf16, tag="s_b")
                nc.vector.scalar_tensor_tensor(
                    out=s_b[:], in0=e1, scalar=-1.0, in1=e2,
                    op0=OP.add, op1=OP.mult,
                    accum_out=A12A[:, i : i + 1],
                )

            nt = n_tiles
            nc.sync.dma_start(out=OUT[:, 0 * nt : 1 * nt], in_=ZMIX[:])
            nc.sync.dma_start(out=OUT[:, 1 * nt : 2 * nt], in_=Z1R[:])
            nc.sync.dma_start(out=OUT[:, 2 * nt : 3 * nt], in_=S11A[:])
            nc.sync.dma_start(out=OUT[:, 3 * nt : 4 * nt], in_=A12A[:])
            nc.sync.dma_start(out=OUT[:, 4 * nt : 5 * nt], in_=S22A[:])

    nc.compile()
    return nc


def _get_program():
    key = "full"
    if key not in _PROG_CACHE:
        _PROG_CACHE[key] = build_program()
    return _PROG_CACHE[key]


def _host_tail(outs: list) -> float:
    nt = N_TILES
    total = 0.0
    for o in outs:
        o = o.astype(np.float64)
        zmix = o[:, 0 * nt : 1 * nt]
        z1 = o[:, 1 * nt : 2 * nt]
        s11a = o[:, 2 * nt : 3 * nt]
        a12 = o[:, 3 * nt : 4 * nt]
        s22 = o[:, 4 * nt : 5 * nt]
        z2 = zmix - 2.0 * C - z1
        s11 = s11a - z1
        s12 = a12 - z1
        num = s12 - z1 * z2 / C
        b = s11 - z1 * z1 / C
        c = s22 - z2 * z2 / C
        total += (num / np.sqrt(b * c)).sum()
    return SCALE * total / float(N_ROWS)


def run_sharded(outputs1: np.ndarray, outputs2: np.ndarray, trace: bool = False):
    from concourse.bass_utils import run_bass_kernel_spmd

    nc = _get_program()
    n = outputs1.shape[0]
    xm = np.empty((n, 2, C), dtype=np.float16)
    xm[:, 0, :] = outputs1
    xm[:, 1, :] = outputs2
    in_maps = [
        {"xm": xm[i * ROWS_PER_CORE : (i + 1) * ROWS_PER_CORE]}
        for i in range(N_CORES)
    ]
    res = run_bass_kernel_spmd(nc, in_maps, list(range(N_CORES)), trace=trace)
    val = _host_tail([r["out"] for r in res.results])
    return np.asarray(val, dtype=np.float32), res


def kernel(outputs1, outputs2, targets=None, **_unused):
    val, _ = run_sharded(np.asarray(outputs1), np.asarray(outputs2))
    return val
